# revision 3
# baseline (speedup 1.0000x reference)
"""Masked 3-layer MLP (tanh) on 8 Trainium2 NeuronCores.

Reference computation (B=2048, dims 4096->8192->8192->4096, fp32):
    h1 = tanh(x @ (W1*m1).T + b1)
    h2 = tanh(h1 @ (W2*m2).T + b2)
    out =      h2 @ (W3*m3).T + b3

The masks are p=1e-4 Bernoulli, so the effective network is tiny. Fast
path ("packed"): core k owns output rows [k*512, (k+1)*512). Walking the
masks backwards, those rows touch only the h2 features S3_k = nonzero
columns of m3[rows_k] (~450), which touch only h1 features S2_k (~380),
which touch only x dims S1_k (~160). The host gathers, for each core, the
masked weight submatrices over exactly those index sets (padded with zeros
to shared multiples of 128), and each core runs a fully LOCAL dense
3-layer MLP with contractions 256->384->512 instead of 4096->8192->8192.
No collectives, no DRAM intermediates: weights, x-pack and both hidden
activations stay SBUF-resident; only the final [512, B] fp32 shard is
written out. Compute is in transposed orientation [features, batch] so
output features land on PSUM partitions and per-partition bias + tanh
fuse into the ScalarE PSUM eviction.

Fallback (masks not sparse enough to pack): the previous Megatron-style
column-parallel dense kernel with on-chip AllGathers after layers 1/2.
"""

import os
import sys

import numpy as np

for _p in ("/opt/trn_rl_repo", os.path.expanduser("~/.axon_site/_ro/trn_rl_repo")):
    if os.path.isdir(_p) and _p not in sys.path:
        sys.path.append(_p)

B = 2048
DIMS = [4096, 8192, 8192, 4096]
NCORES = 8
P = 128
FD = 512           # matmul moving free dim == one PSUM bank of fp32
NB = B // FD       # batch blocks
ICK = 4            # K-subtiles (x128 rows) per streamed input chunk
MCK = 4            # K-subtiles per weight/mask load+mask chunk

# Compute dtype: fp16 | bf16 | fp32r | fp32
DTYPE = os.environ.get("BASS_MLP_DTYPE", "fp16")
# Output dtype of the device kernel ("fp32" safe; "cdt" halves out DMA)
OUT_DT = os.environ.get("BASS_MLP_OUT_DT", "fp32")

_cache = {}


def _np_cdt():
    if DTYPE == "bf16":
        import ml_dtypes

        return ml_dtypes.bfloat16
    return {"fp16": np.float16, "fp32r": np.float32, "fp32": np.float32}[DTYPE]


# --------------------------------------------------------------------------
# Packed (sparse-mask) fast path
# --------------------------------------------------------------------------

PACK_MAX = 1024    # per-layer packed contraction cap (SBUF/PSUM budget)


def _rup(n, m=P):
    return max(m, (n + m - 1) // m * m)


def plan_packed(m1, m2, m3):
    """Per-core index sets walking the masks backwards from the output
    shard. Returns (sizes (K1, F1, F2), per-core (S1, S2, S3)) or None if
    any packed dim exceeds PACK_MAX."""
    m1 = np.asarray(m1)
    m2 = np.asarray(m2)
    m3 = np.asarray(m3)
    fs3 = DIMS[3] // NCORES
    idxs = []
    k1 = f1 = f2 = 0
    for k in range(NCORES):
        S3 = np.flatnonzero(m3[k * fs3:(k + 1) * fs3].any(axis=0))
        S2 = np.flatnonzero(m2[S3].any(axis=0))
        S1 = np.flatnonzero(m1[S2].any(axis=0))
        if len(S3) > PACK_MAX or len(S2) > PACK_MAX or len(S1) > PACK_MAX:
            return None
        idxs.append((S1, S2, S3))
        k1, f1, f2 = max(k1, len(S1)), max(f1, len(S2)), max(f2, len(S3))
    return (_rup(k1), _rup(f1), _rup(f2)), idxs


def _build_packed(k1, f1, f2):
    """Single-core-local packed MLP: [k1]->[f1]->[f2]->[512], B=2048.
    Same NEFF on all 8 cores; per-core inputs differ. No collectives."""
    import concourse.tile as tile
    from concourse import bacc, mybir
    from concourse.bass import DynSlice

    cdt = {
        "fp16": mybir.dt.float16,
        "bf16": mybir.dt.bfloat16,
        "fp32r": mybir.dt.float32r,
        "fp32": mybir.dt.float32,
    }[DTYPE]
    odt = cdt if OUT_DT == "cdt" else mybir.dt.float32

    f3 = DIMS[3] // NCORES                     # 512 output rows per core
    KS = [k1, f1, f2]                          # contraction per layer
    FS = [f1, f2, f3]                          # output features per layer

    nc = bacc.Bacc(None, target_bir_lowering=False, debug=False,
                   num_devices=NCORES)

    xp = nc.dram_tensor("xp", [k1, B], cdt, kind="ExternalInput")
    wts, bs = [], []
    for li in range(3):
        wts.append(nc.dram_tensor(f"w{li + 1}p", [KS[li], FS[li]], cdt,
                                  kind="ExternalInput"))
        bs.append(nc.dram_tensor(f"b{li + 1}p", [FS[li]], mybir.dt.float32,
                                 kind="ExternalInput"))
    out = nc.dram_tensor("out", [f3, B], odt, kind="ExternalOutput")

    with tile.TileContext(nc) as tc:
        with tc.tile_pool(name="per", bufs=1) as per, \
             tc.tile_pool(name="op", bufs=6) as opool, \
             tc.tile_pool(name="ps", bufs=8, space="PSUM") as pspool:

            # ---- persistent SBUF residents ----
            xt = per.tile([P, k1 // P, B], cdt, tag="xt", name="xt")
            wt = [per.tile([P, KS[li] // P, FS[li]], cdt, tag=f"w{li}",
                           name=f"wt{li}") for li in range(3)]
            bt = [per.tile([P, FS[li] // P], mybir.dt.float32, tag=f"b{li}",
                           name=f"bt{li}") for li in range(3)]
            h = [per.tile([P, FS[li] // P, B], cdt, tag=f"h{li}",
                          name=f"ht{li}") for li in range(2)]

            # input DMAs spread over independent queues
            nc.sync.dma_start(xt[:], xp.ap().rearrange(
                "(ko p) n -> p ko n", p=P))
            qs = [nc.scalar, nc.gpsimd, nc.scalar]
            for li in range(3):
                qs[li].dma_start(wt[li][:], wts[li].ap().rearrange(
                    "(ko p) f -> p ko f", p=P))
                qs[li].dma_start(bt[li][:], bs[li].ap().rearrange(
                    "(o p) -> p o", p=P))

            oqs = [nc.sync, nc.scalar, nc.gpsimd]
            for li in range(3):
                K, F = KS[li], FS[li]
                KO, NF = K // P, F // P
                src = xt if li == 0 else h[li - 1]
                tanh = li < 2
                func = (mybir.ActivationFunctionType.Tanh if tanh
                        else mybir.ActivationFunctionType.Identity)
                for b in range(NB):
                    nsl = DynSlice(b * FD, FD)
                    for f in range(NF):
                        ps = pspool.tile([P, FD], mybir.dt.float32, tag="ps",
                                         name=f"ps{li}_{b}_{f}")
                        for ko in range(KO):
                            nc.tensor.matmul(
                                ps[:],
                                wt[li][:, ko, DynSlice(f * P, P)],
                                src[:, ko, nsl],
                                start=(ko == 0), stop=(ko == KO - 1))
                        if li < 2:
                            nc.scalar.activation(
                                h[li][:, f, nsl], ps[:], func,
                                bias=bt[li][:, DynSlice(f, 1)])
                        else:
                            ot = opool.tile([P, FD], odt, tag="prod",
                                            name=f"o{b}_{f}")
                            nc.scalar.activation(
                                ot[:], ps[:], func,
                                bias=bt[li][:, DynSlice(f, 1)])
                            oqs[(b * NF + f) % len(oqs)].dma_start(
                                out.ap()[DynSlice(f * P, P), nsl], ot[:])

    nc.compile()
    return nc


def make_in_maps_packed(x, W1, b1, m1, W2, b2, m2, W3, b3, m3, sizes, idxs):
    k1, f1, f2 = sizes
    f3 = DIMS[3] // NCORES
    npdt = _np_cdt()
    x, W1, b1, m1, W2, b2, m2, W3, b3, m3 = (
        np.asarray(a) for a in (x, W1, b1, m1, W2, b2, m2, W3, b3, m3))
    in_maps = []
    for k in range(NCORES):
        S1, S2, S3 = idxs[k]
        rows = slice(k * f3, (k + 1) * f3)
        m = {}
        xk = np.zeros((k1, B), npdt)
        xk[:len(S1)] = x[:, S1].T
        m["xp"] = xk

        w1 = np.zeros((k1, f1), npdt)
        w1[:len(S1), :len(S2)] = (W1[np.ix_(S2, S1)] * m1[np.ix_(S2, S1)]).T
        m["w1p"] = w1
        w2 = np.zeros((f1, f2), npdt)
        w2[:len(S2), :len(S3)] = (W2[np.ix_(S3, S2)] * m2[np.ix_(S3, S2)]).T
        m["w2p"] = w2
        w3 = np.zeros((f2, f3), npdt)
        w3[:len(S3)] = (W3[rows][:, S3] * m3[rows][:, S3]).T
        m["w3p"] = w3

        bp1 = np.zeros(f1, np.float32)
        bp1[:len(S2)] = b1[S2]
        m["b1p"] = bp1
        bp2 = np.zeros(f2, np.float32)
        bp2[:len(S3)] = b2[S3]
        m["b2p"] = bp2
        m["b3p"] = np.ascontiguousarray(b3[rows]).astype(np.float32)
        in_maps.append(m)
    return in_maps


# --------------------------------------------------------------------------
# Dense fallback (Megatron column-parallel, AllGather after layers 1/2)
# --------------------------------------------------------------------------

def _build(l1k=DIMS[0]):
    """Build + schedule the SPMD Bass program (same NEFF on all 8 cores).

    l1k: layer-1 contraction size. DIMS[0] for the dense path; a smaller
    multiple of 512 when the host packs only the K-rows that survive m1
    (per-core), padding with zeros.
    """
    import concourse.tile as tile
    from concourse import bacc, mybir
    from concourse.bass import DynSlice

    cdt = {
        "fp16": mybir.dt.float16,
        "bf16": mybir.dt.bfloat16,
        "fp32r": mybir.dt.float32r,  # rounded fp32; np side is float32
        "fp32": mybir.dt.float32,
    }[DTYPE]
    esz = mybir.dt.size(cdt)

    # Per-layer output-feature shard sizes and weight-panel widths.
    FS = [DIMS[1] // NCORES, DIMS[2] // NCORES, DIMS[3] // NCORES]  # 1024,1024,512
    KS = [l1k, DIMS[1], DIMS[2]]
    if esz == 2:
        # Uniform 64KB/partition weight-panel slots so wpool can double-buffer:
        # the next panel's DMA+mask overlaps the current panel's matmuls.
        FBLK = [1024, 512, 512]
        mck, ibufs, wbufs = MCK, 6, 2
    else:
        FBLK = [1024, 512, 512]      # L2 split into two panels (SBUF)
        mck, ibufs, wbufs = 2, 4, 1

    nc = bacc.Bacc(None, target_bir_lowering=False, debug=False, num_devices=NCORES)

    xT = nc.dram_tensor("xT", [KS[0], B], cdt, kind="ExternalInput")
    wts, mts, bs = [], [], []
    for li in range(3):
        wts.append(nc.dram_tensor(f"w{li + 1}t", [KS[li], FS[li]], cdt,
                                  kind="ExternalInput"))
        mts.append(nc.dram_tensor(f"m{li + 1}t", [KS[li], FS[li]], cdt,
                                  kind="ExternalInput"))
        bs.append(nc.dram_tensor(f"b{li + 1}", [FS[li]], mybir.dt.float32,
                                 kind="ExternalInput"))
    out = nc.dram_tensor("out", [FS[2], B], mybir.dt.float32,
                         kind="ExternalOutput")

    with tile.TileContext(nc) as tc:
        with tc.tile_pool(name="wp", bufs=wbufs) as wpool, \
             tc.tile_pool(name="inp", bufs=ibufs) as ipool, \
             tc.tile_pool(name="mp", bufs=2) as mpool, \
             tc.tile_pool(name="op", bufs=6) as opool, \
             tc.tile_pool(name="bp", bufs=3) as bpool, \
             tc.tile_pool(name="ps", bufs=8, space="PSUM") as pspool, \
             tc.tile_pool(name="dram", bufs=1, space="DRAM") as dram:

            # Per-(layer, b-block) activation tensors so each AllGather covers
            # one 512-batch block and pipelines behind compute.
            h_loc = [[dram.tile([FS[li], FD], cdt, name=f"h{li + 1}_loc{b}")
                      for b in range(NB)] for li in range(2)]
            h_full = [[dram.tile([DIMS[li + 1], FD], cdt, addr_space="Shared",
                                 name=f"h{li + 1}_full{b}")
                       for b in range(NB)] for li in range(2)]

            def layer(li, tanh):
                K, F = KS[li], FS[li]
                KO = K // P
                wt_r = wts[li].ap().rearrange("(ko p) f -> p ko f", p=P)
                mt_r = mts[li].ap().rearrange("(ko p) f -> p ko f", p=P)
                if li == 0:
                    xr = xT.ap().rearrange("(ko p) n -> p ko n", p=P)
                    in_rs = [xr[:, :, DynSlice(b * FD, FD)] for b in range(NB)]
                else:
                    in_rs = [h_full[li - 1][b][:].rearrange(
                        "(ko p) n -> p ko n", p=P) for b in range(NB)]

                btile = bpool.tile([P, F // P], mybir.dt.float32, tag="bias",
                                   name=f"bias{li}")
                nc.sync.dma_start(btile[:], bs[li].ap().rearrange(
                    "(o p) -> p o", p=P))

                fblk = FBLK[li]
                for f0 in range(0, F, fblk):
                    # --- load + mask one weight panel [P, KO, fblk] ---
                    wp = wpool.tile([P, KO, fblk], cdt, tag="wpanel",
                                    name=f"wp{li}_{f0}")
                    # weight/mask loads go on gpsimd/vector DMA queues so the
                    # input-strip stream on the sync queue is never stuck
                    # behind a 16MB panel load
                    for c0 in range(0, KO, mck):
                        csl = slice(c0, c0 + mck)
                        fsl = DynSlice(f0, fblk)
                        nc.gpsimd.dma_start(wp[:, csl, :], wt_r[:, csl, fsl])
                        mtile = mpool.tile([P, mck, fblk], cdt, tag="mchunk",
                                           name=f"m{li}_{f0}_{c0}")
                        nc.gpsimd.dma_start(mtile[:], mt_r[:, csl, fsl])
                        nc.vector.tensor_tensor(wp[:, csl, :], wp[:, csl, :],
                                                mtile[:], mybir.AluOpType.mult)

                    nf = fblk // P
                    for b in range(NB):
                        psums = [pspool.tile([P, FD], mybir.dt.float32,
                                             tag="ps", name=f"ps{li}_{f0}_{b}_{f}")
                                 for f in range(nf)]
                        for c0 in range(0, KO, ICK):
                            it = ipool.tile([P, ICK, FD], cdt, tag="instrip",
                                            name=f"in{li}_{f0}_{b}_{c0}")
                            nc.sync.dma_start(
                                it[:], in_rs[b][:, slice(c0, c0 + ICK), :])
                            for f in range(nf):
                                for ks in range(ICK):
                                    ko = c0 + ks
                                    nc.tensor.matmul(
                                        psums[f][:],
                                        wp[:, ko, DynSlice(f * P, P)],
                                        it[:, ks, :],
                                        start=(ko == 0), stop=(ko == KO - 1))
                        for f in range(nf):
                            fg = f0 + f * P   # feature row offset in shard
                            odt = cdt if li < 2 else mybir.dt.float32
                            ot = opool.tile([P, FD], odt, tag="prod",
                                            name=f"o{li}_{f0}_{b}_{f}")
                            func = (mybir.ActivationFunctionType.Tanh if tanh
                                    else mybir.ActivationFunctionType.Identity)
                            nc.scalar.activation(
                                ot[:], psums[f][:], func,
                                bias=btile[:, DynSlice((f0 // P) + f, 1)])
                            if li < 2:
                                nc.sync.dma_start(
                                    h_loc[li][b][DynSlice(fg, P), :], ot[:])
                            else:
                                nc.sync.dma_start(
                                    out.ap()[DynSlice(fg, P),
                                             DynSlice(b * FD, FD)], ot[:])
                        # fire this b-block's AllGather as soon as the last
                        # panel has written it
                        if li < 2 and f0 == F - fblk:
                            nc.gpsimd.collective_compute(
                                "AllGather",
                                mybir.AluOpType.bypass,
                                replica_groups=[list(range(NCORES))],
                                ins=[h_loc[li][b].opt()],
                                outs=[h_full[li][b].opt()],
                            )

            layer(0, tanh=True)
            layer(1, tanh=True)
            layer(2, tanh=False)

    nc.compile()
    return nc


PACK_K = 512   # packed layer-1 contraction size (dense-path fast variant)


def get_nc(l1k=DIMS[0]):
    key = ("dense", l1k)
    if key not in _cache:
        _cache[key] = _build(l1k)
    return _cache[key]


def get_nc_packed(sizes):
    key = ("packed", sizes)
    if key not in _cache:
        _cache[key] = _build_packed(*sizes)
    return _cache[key]


def plan_l1k(m1):
    """If m1 is sparse enough that every core's shard of (W1*m1).T touches at
    most PACK_K input dims, return (PACK_K, per-core used-row indices); else
    the dense plan."""
    m1 = np.asarray(m1)
    fs = DIMS[1] // NCORES
    idxs = []
    for k in range(NCORES):
        idx = np.flatnonzero(m1[k * fs:(k + 1) * fs].any(axis=0))
        if len(idx) > PACK_K:
            return DIMS[0], None
        idxs.append(idx)
    return PACK_K, idxs


def make_in_maps(x, W1, b1, m1, W2, b2, m2, W3, b3, m3, idxs=None):
    """Host-side sharding: transpose to [K, F] layouts, cast, slice shards.
    With idxs, layer-1 operands are gathered to the PACK_K used K-rows."""
    x, W1, b1, m1, W2, b2, m2, W3, b3, m3 = (
        np.asarray(a) for a in (x, W1, b1, m1, W2, b2, m2, W3, b3, m3))
    npdt = _np_cdt()
    xT = np.ascontiguousarray(x.T).astype(npdt, copy=False)
    Ws = [W1, W2, W3]
    Ms = [m1, m2, m3]
    Bs = [b1, b2, b3]
    in_maps = []
    for k in range(NCORES):
        m = {}
        for li in range(3):
            F = DIMS[li + 1]
            fs = F // NCORES
            sl = slice(k * fs, (k + 1) * fs)
            wt = Ws[li][sl].T
            mt = Ms[li][sl].T
            if li == 0:
                if idxs is None:
                    m["xT"] = xT
                else:
                    idx = idxs[k]
                    xk = np.zeros((PACK_K, B), npdt)
                    xk[:len(idx)] = xT[idx]
                    m["xT"] = xk
                    wk = np.zeros((PACK_K, fs), npdt)
                    wk[:len(idx)] = wt[idx].astype(npdt)
                    mk = np.zeros((PACK_K, fs), npdt)
                    mk[:len(idx)] = mt[idx].astype(npdt)
                    m["w1t"], m["m1t"] = wk, mk
            if f"w{li + 1}t" not in m:
                m[f"w{li + 1}t"] = np.ascontiguousarray(wt).astype(
                    npdt, copy=False)
                m[f"m{li + 1}t"] = np.ascontiguousarray(mt).astype(npdt)
            m[f"b{li + 1}"] = np.ascontiguousarray(Bs[li][sl]).astype(
                np.float32, copy=False)
        in_maps.append(m)
    return in_maps


def prepare(x, W1, b1, m1, W2, b2, m2, W3, b3, m3):
    """Plan, build (cached), and shard: returns (nc, in_maps)."""
    plan = plan_packed(m1, m2, m3)
    if plan is not None:
        sizes, idxs = plan
        nc = get_nc_packed(sizes)
        in_maps = make_in_maps_packed(x, W1, b1, m1, W2, b2, m2, W3, b3, m3,
                                      sizes, idxs)
    else:
        l1k, idxs = plan_l1k(m1)
        nc = get_nc(l1k)
        in_maps = make_in_maps(x, W1, b1, m1, W2, b2, m2, W3, b3, m3,
                               idxs=idxs)
    return nc, in_maps


def kernel(x, W1, b1, m1, W2, b2, m2, W3, b3, m3):
    from concourse.bass_utils import run_bass_kernel_spmd

    nc, in_maps = prepare(x, W1, b1, m1, W2, b2, m2, W3, b3, m3)
    res = run_bass_kernel_spmd(nc, in_maps, core_ids=list(range(NCORES)))
    outT = np.concatenate([res.results[k]["out"] for k in range(NCORES)],
                          axis=0).astype(np.float32, copy=False)
    return np.ascontiguousarray(outT.T)


# revision 11
# speedup vs baseline: 1.0543x; 1.0543x over previous
"""Masked 3-layer MLP (tanh) on 8 Trainium2 NeuronCores.

Reference computation (B=2048, dims 4096->8192->8192->4096, fp32):
    h1 = tanh(x @ (W1*m1).T + b1)
    h2 = tanh(h1 @ (W2*m2).T + b2)
    out =      h2 @ (W3*m3).T + b3

The masks are p=1e-4 Bernoulli, so the effective network is tiny. Fast
path ("packed"): core k owns output rows [k*512, (k+1)*512). Walking the
masks backwards, those rows touch only the h2 features S3_k = nonzero
columns of m3[rows_k] (~450), which touch only h1 features S2_k (~380),
which touch only x dims S1_k (~160). The host gathers, for each core, the
masked weight submatrices over exactly those index sets (padded with zeros
to shared multiples of 128), and each core runs a fully LOCAL dense
3-layer MLP with contractions 256->384->512 instead of 4096->8192->8192.
No collectives, no DRAM intermediates: weights, x-pack and both hidden
activations stay SBUF-resident; only the final [512, B] fp32 shard is
written out. Compute is in transposed orientation [features, batch] so
output features land on PSUM partitions and per-partition bias + tanh
fuse into the ScalarE PSUM eviction.

Fallback (masks not sparse enough to pack): the previous Megatron-style
column-parallel dense kernel with on-chip AllGathers after layers 1/2.
"""

import os
import sys

import numpy as np

for _p in ("/opt/trn_rl_repo", os.path.expanduser("~/.axon_site/_ro/trn_rl_repo")):
    if os.path.isdir(_p) and _p not in sys.path:
        sys.path.append(_p)

B = 2048
DIMS = [4096, 8192, 8192, 4096]
NCORES = 8
P = 128
FD = 512           # matmul moving free dim == one PSUM bank of fp32
NB = B // FD       # batch blocks
ICK = 4            # K-subtiles (x128 rows) per streamed input chunk
MCK = 4            # K-subtiles per weight/mask load+mask chunk

# Compute dtype: fp16 | bf16 | fp32r | fp32
DTYPE = os.environ.get("BASS_MLP_DTYPE", "fp16")
# Output dtype of the device kernel ("fp32" safe; "cdt" halves out DMA)
OUT_DT = os.environ.get("BASS_MLP_OUT_DT", "cdt")
# tanh(SAT) == 1.0 exactly after rounding to the compute dtype; used to
# synthesize constant-1 pad rows that carry the biases through the matmuls
SAT = 30.0

_cache = {}


def _np_cdt():
    if DTYPE == "bf16":
        import ml_dtypes

        return ml_dtypes.bfloat16
    return {"fp16": np.float16, "fp32r": np.float32, "fp32": np.float32}[DTYPE]


# --------------------------------------------------------------------------
# Packed (sparse-mask) fast path
# --------------------------------------------------------------------------

PACK_MAX = 1024    # per-layer packed contraction cap (SBUF/PSUM budget)


def _rup(n, m=P):
    return max(m, (n + m - 1) // m * m)


def plan_packed(m1, m2, m3):
    """Per-core index sets walking the masks backwards from the output
    shard. Returns (sizes (K1, F1, F2), per-core (S1, S2, S3)) or None if
    any packed dim exceeds PACK_MAX. Sizes reserve one extra row per dim
    for the constant-1 bias-carrier row."""
    m1 = np.asarray(m1)
    m2 = np.asarray(m2)
    m3 = np.asarray(m3)
    fs3 = DIMS[3] // NCORES
    idxs = []
    k1 = f1 = f2 = 0
    for k in range(NCORES):
        S3 = np.flatnonzero(m3[k * fs3:(k + 1) * fs3].any(axis=0))
        S2 = np.flatnonzero(m2[S3].any(axis=0))
        S1 = np.flatnonzero(m1[S2].any(axis=0))
        if len(S3) > PACK_MAX or len(S2) > PACK_MAX or len(S1) > PACK_MAX:
            return None
        idxs.append((S1, S2, S3))
        k1, f1, f2 = max(k1, len(S1)), max(f1, len(S2)), max(f2, len(S3))
    return (_rup(k1 + 1), _rup(f1 + 1), _rup(f2 + 1)), idxs


def _build_packed(k1, f1, f2):
    """Single-core-local packed MLP: [k1]->[f1]->[f2]->[512], B=2048.
    Same NEFF on all 8 cores; per-core inputs differ. No collectives.

    Biases ride inside the weight matrices via constant-1 pad rows (the
    host sets xp[k1-1]=1 and chains tanh(SAT)=1 carriers through h1/h2),
    so PSUM evictions are bias-free and can cover two PSUM banks at once:
    ScalarE does [128,1024] tanh for layers 1/2, the otherwise-idle DVE
    does the [128,1024] cast/copy for layer 3. Matmuls run with batch
    innermost so one stationary Ldweights serves 4 moving blocks."""
    import concourse.tile as tile
    from concourse import bacc, mybir
    from concourse.bass import DynSlice

    cdt = {
        "fp16": mybir.dt.float16,
        "bf16": mybir.dt.bfloat16,
        "fp32r": mybir.dt.float32r,
        "fp32": mybir.dt.float32,
    }[DTYPE]
    odt = cdt if OUT_DT == "cdt" else mybir.dt.float32

    f3 = DIMS[3] // NCORES                     # 512 output rows per core
    KS = [k1, f1, f2]                          # contraction per layer
    FS = [f1, f2, f3]                          # output features per layer

    nc = bacc.Bacc(None, target_bir_lowering=False, debug=False,
                   num_devices=NCORES)

    xp = nc.dram_tensor("xp", [k1, B], cdt, kind="ExternalInput")
    wts = [nc.dram_tensor(f"w{li + 1}p", [KS[li], FS[li]], cdt,
                          kind="ExternalInput") for li in range(3)]
    out = nc.dram_tensor("out", [f3, B], odt, kind="ExternalOutput")

    with tile.TileContext(nc) as tc:
        with tc.tile_pool(name="per", bufs=1) as per, \
             tc.tile_pool(name="op", bufs=4) as opool, \
             tc.tile_pool(name="ps", bufs=4, space="PSUM") as pspool:

            # ---- persistent SBUF residents ----
            xtb = [per.tile([P, k1 // P, FD], cdt, tag=f"xt{b}",
                            name=f"xt{b}") for b in range(NB)]
            wt = [per.tile([P, KS[li] // P, FS[li]], cdt, tag=f"w{li}",
                           name=f"wt{li}") for li in range(3)]
            h = [per.tile([P, FS[li] // P, B], cdt, tag=f"h{li}",
                          name=f"ht{li}") for li in range(2)]

            # Input DMAs: ALL on the sync queue, ordered by first use — the
            # SDMA engines drain one transfer at a time in descriptor-
            # arrival order, and a single queue is the only way to control
            # that order (two HWDGE queues interleave descriptor-gen). xp
            # is chunked per batch block so the first matmul starts after
            # ~256KB, not 1MB. Output DMAs stay off the sync queue's head
            # and off the ACT/DVE queues (a blocked dma_start would
            # head-of-line-block the engine's own compute dispatch).
            xr = xp.ap().rearrange("(ko p) n -> p ko n", p=P)
            wr = [wts[li].ap().rearrange("(ko p) f -> p ko f", p=P)
                  for li in range(3)]
            nc.sync.dma_start(wt[0][:], wr[0])
            for b in range(NB):
                nc.sync.dma_start(xtb[b][:], xr[:, :, DynSlice(b * FD, FD)])
            nc.sync.dma_start(wt[1][:], wr[1])
            nc.sync.dma_start(wt[2][:], wr[2])

            oqs = [nc.scalar, nc.gpsimd, nc.sync]
            oq = 0
            for li in range(3):
                K, F = KS[li], FS[li]
                KO, NF = K // P, F // P
                tanh = li < 2
                for f in range(NF):
                    wsl = DynSlice(f * P, P)
                    # two [128,1024] psum tiles (= 2 banks each): halves
                    # hold batch pairs (b0,b1) and (b2,b3)
                    pps = [pspool.tile([P, 2 * FD], mybir.dt.float32,
                                       tag="ps", name=f"ps{li}_{f}_{hf}")
                           for hf in range(2)]
                    for ko in range(KO):
                        wap = wt[li][:, ko, wsl]
                        for b in range(NB):
                            src = (xtb[b][:, ko, :] if li == 0 else
                                   h[li - 1][:, ko, DynSlice(b * FD, FD)])
                            nc.tensor.matmul(
                                pps[b // 2][:, DynSlice((b % 2) * FD, FD)],
                                wap, src,
                                start=(ko == 0), stop=(ko == KO - 1))
                    if li < 2:
                        for hf in range(2):
                            osl = DynSlice(hf * 2 * FD, 2 * FD)
                            nc.scalar.activation(
                                h[li][:, f, osl], pps[hf][:],
                                mybir.ActivationFunctionType.Tanh)
                    elif f < NF - 1:
                        for hf in range(2):
                            osl = DynSlice(hf * 2 * FD, 2 * FD)
                            ot = opool.tile([P, 2 * FD], odt, tag="prod",
                                            name=f"o{f}_{hf}")
                            nc.vector.tensor_scalar_add(ot[:], pps[hf][:], 0.0)
                            oqs[oq % len(oqs)].dma_start(
                                out.ap()[wsl, osl], ot[:])
                            oq += 1
                    else:
                        # last output group is the kernel tail: evict in
                        # [128,512] quarters on both DVE and ACT in
                        # parallel, out-DMAs fanned across all queues
                        for q in range(4):
                            osl = DynSlice(q * FD, FD)
                            ot = opool.tile([P, FD], odt, tag="prodq",
                                            name=f"oq{q}")
                            psl = pps[q // 2][:, DynSlice((q % 2) * FD, FD)]
                            if q % 2 == 0:
                                nc.vector.tensor_scalar_add(ot[:], psl, 0.0)
                            else:
                                nc.scalar.activation(
                                    ot[:], psl,
                                    mybir.ActivationFunctionType.Copy)
                            oqs[(oq + q) % len(oqs)].dma_start(
                                out.ap()[wsl, osl], ot[:])

    nc.compile()
    return nc


def make_in_maps_packed(x, W1, b1, m1, W2, b2, m2, W3, b3, m3, sizes, idxs):
    """Gather per-core packed submatrices; fold biases in via constant-1
    carrier rows: xp[k1-1]=1 carries b1 (w1p[k1-1]) and seeds h1[f1-1]=
    tanh(SAT)=1, which carries b2 (w2p[f1-1]) and seeds h2[f2-1]=1, which
    carries b3 (w3p[f2-1])."""
    k1, f1, f2 = sizes
    f3 = DIMS[3] // NCORES
    npdt = _np_cdt()
    x, W1, b1, m1, W2, b2, m2, W3, b3, m3 = (
        np.asarray(a) for a in (x, W1, b1, m1, W2, b2, m2, W3, b3, m3))
    in_maps = []
    for k in range(NCORES):
        S1, S2, S3 = idxs[k]
        rows = slice(k * f3, (k + 1) * f3)
        m = {}
        xk = np.zeros((k1, B), npdt)
        xk[:len(S1)] = x[:, S1].T
        xk[k1 - 1] = 1.0
        m["xp"] = xk

        w1 = np.zeros((k1, f1), npdt)
        w1[:len(S1), :len(S2)] = (W1[np.ix_(S2, S1)] * m1[np.ix_(S2, S1)]).T
        w1[k1 - 1, :len(S2)] = b1[S2]
        w1[k1 - 1, f1 - 1] = SAT
        m["w1p"] = w1
        w2 = np.zeros((f1, f2), npdt)
        w2[:len(S2), :len(S3)] = (W2[np.ix_(S3, S2)] * m2[np.ix_(S3, S2)]).T
        w2[f1 - 1, :len(S3)] = b2[S3]
        w2[f1 - 1, f2 - 1] = SAT
        m["w2p"] = w2
        w3 = np.zeros((f2, f3), npdt)
        w3[:len(S3)] = (W3[rows][:, S3] * m3[rows][:, S3]).T
        w3[f2 - 1] = b3[rows]
        m["w3p"] = w3
        in_maps.append(m)
    return in_maps


# --------------------------------------------------------------------------
# Dense fallback (Megatron column-parallel, AllGather after layers 1/2)
# --------------------------------------------------------------------------

def _build(l1k=DIMS[0]):
    """Build + schedule the SPMD Bass program (same NEFF on all 8 cores).

    l1k: layer-1 contraction size. DIMS[0] for the dense path; a smaller
    multiple of 512 when the host packs only the K-rows that survive m1
    (per-core), padding with zeros.
    """
    import concourse.tile as tile
    from concourse import bacc, mybir
    from concourse.bass import DynSlice

    cdt = {
        "fp16": mybir.dt.float16,
        "bf16": mybir.dt.bfloat16,
        "fp32r": mybir.dt.float32r,  # rounded fp32; np side is float32
        "fp32": mybir.dt.float32,
    }[DTYPE]
    esz = mybir.dt.size(cdt)

    # Per-layer output-feature shard sizes and weight-panel widths.
    FS = [DIMS[1] // NCORES, DIMS[2] // NCORES, DIMS[3] // NCORES]  # 1024,1024,512
    KS = [l1k, DIMS[1], DIMS[2]]
    if esz == 2:
        # Uniform 64KB/partition weight-panel slots so wpool can double-buffer:
        # the next panel's DMA+mask overlaps the current panel's matmuls.
        FBLK = [1024, 512, 512]
        mck, ibufs, wbufs = MCK, 6, 2
    else:
        FBLK = [1024, 512, 512]      # L2 split into two panels (SBUF)
        mck, ibufs, wbufs = 2, 4, 1

    nc = bacc.Bacc(None, target_bir_lowering=False, debug=False, num_devices=NCORES)

    xT = nc.dram_tensor("xT", [KS[0], B], cdt, kind="ExternalInput")
    wts, mts, bs = [], [], []
    for li in range(3):
        wts.append(nc.dram_tensor(f"w{li + 1}t", [KS[li], FS[li]], cdt,
                                  kind="ExternalInput"))
        mts.append(nc.dram_tensor(f"m{li + 1}t", [KS[li], FS[li]], cdt,
                                  kind="ExternalInput"))
        bs.append(nc.dram_tensor(f"b{li + 1}", [FS[li]], mybir.dt.float32,
                                 kind="ExternalInput"))
    out = nc.dram_tensor("out", [FS[2], B], mybir.dt.float32,
                         kind="ExternalOutput")

    with tile.TileContext(nc) as tc:
        with tc.tile_pool(name="wp", bufs=wbufs) as wpool, \
             tc.tile_pool(name="inp", bufs=ibufs) as ipool, \
             tc.tile_pool(name="mp", bufs=2) as mpool, \
             tc.tile_pool(name="op", bufs=6) as opool, \
             tc.tile_pool(name="bp", bufs=3) as bpool, \
             tc.tile_pool(name="ps", bufs=8, space="PSUM") as pspool, \
             tc.tile_pool(name="dram", bufs=1, space="DRAM") as dram:

            # Per-(layer, b-block) activation tensors so each AllGather covers
            # one 512-batch block and pipelines behind compute.
            h_loc = [[dram.tile([FS[li], FD], cdt, name=f"h{li + 1}_loc{b}")
                      for b in range(NB)] for li in range(2)]
            h_full = [[dram.tile([DIMS[li + 1], FD], cdt, addr_space="Shared",
                                 name=f"h{li + 1}_full{b}")
                       for b in range(NB)] for li in range(2)]

            def layer(li, tanh):
                K, F = KS[li], FS[li]
                KO = K // P
                wt_r = wts[li].ap().rearrange("(ko p) f -> p ko f", p=P)
                mt_r = mts[li].ap().rearrange("(ko p) f -> p ko f", p=P)
                if li == 0:
                    xr = xT.ap().rearrange("(ko p) n -> p ko n", p=P)
                    in_rs = [xr[:, :, DynSlice(b * FD, FD)] for b in range(NB)]
                else:
                    in_rs = [h_full[li - 1][b][:].rearrange(
                        "(ko p) n -> p ko n", p=P) for b in range(NB)]

                btile = bpool.tile([P, F // P], mybir.dt.float32, tag="bias",
                                   name=f"bias{li}")
                nc.sync.dma_start(btile[:], bs[li].ap().rearrange(
                    "(o p) -> p o", p=P))

                fblk = FBLK[li]
                for f0 in range(0, F, fblk):
                    # --- load + mask one weight panel [P, KO, fblk] ---
                    wp = wpool.tile([P, KO, fblk], cdt, tag="wpanel",
                                    name=f"wp{li}_{f0}")
                    # weight/mask loads go on gpsimd/vector DMA queues so the
                    # input-strip stream on the sync queue is never stuck
                    # behind a 16MB panel load
                    for c0 in range(0, KO, mck):
                        csl = slice(c0, c0 + mck)
                        fsl = DynSlice(f0, fblk)
                        nc.gpsimd.dma_start(wp[:, csl, :], wt_r[:, csl, fsl])
                        mtile = mpool.tile([P, mck, fblk], cdt, tag="mchunk",
                                           name=f"m{li}_{f0}_{c0}")
                        nc.gpsimd.dma_start(mtile[:], mt_r[:, csl, fsl])
                        nc.vector.tensor_tensor(wp[:, csl, :], wp[:, csl, :],
                                                mtile[:], mybir.AluOpType.mult)

                    nf = fblk // P
                    for b in range(NB):
                        psums = [pspool.tile([P, FD], mybir.dt.float32,
                                             tag="ps", name=f"ps{li}_{f0}_{b}_{f}")
                                 for f in range(nf)]
                        for c0 in range(0, KO, ICK):
                            it = ipool.tile([P, ICK, FD], cdt, tag="instrip",
                                            name=f"in{li}_{f0}_{b}_{c0}")
                            nc.sync.dma_start(
                                it[:], in_rs[b][:, slice(c0, c0 + ICK), :])
                            for f in range(nf):
                                for ks in range(ICK):
                                    ko = c0 + ks
                                    nc.tensor.matmul(
                                        psums[f][:],
                                        wp[:, ko, DynSlice(f * P, P)],
                                        it[:, ks, :],
                                        start=(ko == 0), stop=(ko == KO - 1))
                        for f in range(nf):
                            fg = f0 + f * P   # feature row offset in shard
                            odt = cdt if li < 2 else mybir.dt.float32
                            ot = opool.tile([P, FD], odt, tag="prod",
                                            name=f"o{li}_{f0}_{b}_{f}")
                            func = (mybir.ActivationFunctionType.Tanh if tanh
                                    else mybir.ActivationFunctionType.Identity)
                            nc.scalar.activation(
                                ot[:], psums[f][:], func,
                                bias=btile[:, DynSlice((f0 // P) + f, 1)])
                            if li < 2:
                                nc.sync.dma_start(
                                    h_loc[li][b][DynSlice(fg, P), :], ot[:])
                            else:
                                nc.sync.dma_start(
                                    out.ap()[DynSlice(fg, P),
                                             DynSlice(b * FD, FD)], ot[:])
                        # fire this b-block's AllGather as soon as the last
                        # panel has written it
                        if li < 2 and f0 == F - fblk:
                            nc.gpsimd.collective_compute(
                                "AllGather",
                                mybir.AluOpType.bypass,
                                replica_groups=[list(range(NCORES))],
                                ins=[h_loc[li][b].opt()],
                                outs=[h_full[li][b].opt()],
                            )

            layer(0, tanh=True)
            layer(1, tanh=True)
            layer(2, tanh=False)

    nc.compile()
    return nc


PACK_K = 512   # packed layer-1 contraction size (dense-path fast variant)


def get_nc(l1k=DIMS[0]):
    key = ("dense", l1k)
    if key not in _cache:
        _cache[key] = _build(l1k)
    return _cache[key]


def get_nc_packed(sizes):
    key = ("packed", sizes)
    if key not in _cache:
        _cache[key] = _build_packed(*sizes)
    return _cache[key]


def plan_l1k(m1):
    """If m1 is sparse enough that every core's shard of (W1*m1).T touches at
    most PACK_K input dims, return (PACK_K, per-core used-row indices); else
    the dense plan."""
    m1 = np.asarray(m1)
    fs = DIMS[1] // NCORES
    idxs = []
    for k in range(NCORES):
        idx = np.flatnonzero(m1[k * fs:(k + 1) * fs].any(axis=0))
        if len(idx) > PACK_K:
            return DIMS[0], None
        idxs.append(idx)
    return PACK_K, idxs


def make_in_maps(x, W1, b1, m1, W2, b2, m2, W3, b3, m3, idxs=None):
    """Host-side sharding: transpose to [K, F] layouts, cast, slice shards.
    With idxs, layer-1 operands are gathered to the PACK_K used K-rows."""
    x, W1, b1, m1, W2, b2, m2, W3, b3, m3 = (
        np.asarray(a) for a in (x, W1, b1, m1, W2, b2, m2, W3, b3, m3))
    npdt = _np_cdt()
    xT = np.ascontiguousarray(x.T).astype(npdt, copy=False)
    Ws = [W1, W2, W3]
    Ms = [m1, m2, m3]
    Bs = [b1, b2, b3]
    in_maps = []
    for k in range(NCORES):
        m = {}
        for li in range(3):
            F = DIMS[li + 1]
            fs = F // NCORES
            sl = slice(k * fs, (k + 1) * fs)
            wt = Ws[li][sl].T
            mt = Ms[li][sl].T
            if li == 0:
                if idxs is None:
                    m["xT"] = xT
                else:
                    idx = idxs[k]
                    xk = np.zeros((PACK_K, B), npdt)
                    xk[:len(idx)] = xT[idx]
                    m["xT"] = xk
                    wk = np.zeros((PACK_K, fs), npdt)
                    wk[:len(idx)] = wt[idx].astype(npdt)
                    mk = np.zeros((PACK_K, fs), npdt)
                    mk[:len(idx)] = mt[idx].astype(npdt)
                    m["w1t"], m["m1t"] = wk, mk
            if f"w{li + 1}t" not in m:
                m[f"w{li + 1}t"] = np.ascontiguousarray(wt).astype(
                    npdt, copy=False)
                m[f"m{li + 1}t"] = np.ascontiguousarray(mt).astype(npdt)
            m[f"b{li + 1}"] = np.ascontiguousarray(Bs[li][sl]).astype(
                np.float32, copy=False)
        in_maps.append(m)
    return in_maps


def prepare(x, W1, b1, m1, W2, b2, m2, W3, b3, m3):
    """Plan, build (cached), and shard: returns (nc, in_maps)."""
    plan = plan_packed(m1, m2, m3)
    if plan is not None:
        sizes, idxs = plan
        nc = get_nc_packed(sizes)
        in_maps = make_in_maps_packed(x, W1, b1, m1, W2, b2, m2, W3, b3, m3,
                                      sizes, idxs)
    else:
        l1k, idxs = plan_l1k(m1)
        nc = get_nc(l1k)
        in_maps = make_in_maps(x, W1, b1, m1, W2, b2, m2, W3, b3, m3,
                               idxs=idxs)
    return nc, in_maps


def kernel(x, W1, b1, m1, W2, b2, m2, W3, b3, m3):
    from concourse.bass_utils import run_bass_kernel_spmd

    nc, in_maps = prepare(x, W1, b1, m1, W2, b2, m2, W3, b3, m3)
    res = run_bass_kernel_spmd(nc, in_maps, core_ids=list(range(NCORES)))
    outT = np.concatenate([res.results[k]["out"] for k in range(NCORES)],
                          axis=0).astype(np.float32, copy=False)
    return np.ascontiguousarray(outT.T)
